# revision 13
# baseline (speedup 1.0000x reference)
"""Trainium2 Bass kernel for the CNF reversible backward solve.

Math restructuring (exact, validated in fp64 against the jax reference):

The per-step recursion is tracked purely in H-space (H=256) via
Z = W1 z + bp(s), Yhat = l*(W1 y + be(s)):
    a_even = tanh(inv_l * Yhat)
    Z     += Mz @ a_even  (+ per-step bias delta)      (Mz = -h W1 W2)
    a_odd  = tanh(Z)
    Yhat' += dby + [eps*Yhat + (l-1)*Z_post] + Mz a_odd,  eps = inv_l - 1

All three live states are PSUM accumulations updated by matmuls only:
Z, Yhat, and a third bank B(s) = eps*Yhat_s + (l-1)*(Z_s + dbz_s) that
carries the ~1e-3-scaled coupling correction.  B obeys the pure-matmul
recursion
    B(s+1) = inv_l*B(s) + inv_l(l-1)*Mz a_e + eps*Mz a_o + rank-4 bias
so the correction enters Y as [one bf16 identity matmul of b=bf16(B)] +
[(l-1)Mz @ a_e block matmuls], and B is rebuilt from the same b with
scaled-Mz tables.  The ONLY vector-engine op per step is the bf16 copy
b = bf16(B) at step start; each PSUM bank has exactly ONE reader (Y:
even ACT, Z: odd ACT, B: the copy), which matters because the tile
framework chains same-tile readers across engines in emission order.
The large states never leave fp32 PSUM; everything bf16-routed is
1e-3-scaled, so rounding is negligible (measured 3.2e-4 end to end).

The device streams all activations a_e, a_o to DRAM; the D-space outputs
are exact fp64 host-side postprocessing:
    y_final = c_y y1 + sum_e gamma_e (W2 @ a_e) + c_b b2
    I_final = h (N sum(c) - sum_s c . a_even_s^2),   c = diag(W1 W2)

Sharding: data-parallel, B=256 -> 32 samples on each of 8 cores;
parameters replicated; gather + assembly on host.
"""

import numpy as np
import ml_dtypes
from contextlib import ExitStack

import concourse.bass as bass
import concourse.tile as tile
from concourse import bacc, mybir
from concourse.bass_utils import run_bass_kernel_spmd

# Problem constants (hardcoded per contract)
NCORES = 8
B, D, H = 256, 64, 256
NSTEP = 64
HSTEP = 1.0 / NSTEP
LCOUP = 0.999
INVL = 1.0 / LCOUP
EPS = INVL - 1.0
BS = B // NCORES  # 32 samples per core
BSH = BS
NBLK = H // 128  # 2 h-blocks
FREE = NBLK * BSH  # 64: free size of H-space tiles, layout (blk, sample)
NEVAL = 2 * NSTEP  # 128
NMZ = NBLK * NBLK * 128  # 512 columns per packed Mz table

# uneven out-DMA chunks: small final chunk shortens the post-loop tail
CHUNK_ENDS = [16, 32, 48, 62, 64]
CHUNK_STARTS = [0] + CHUNK_ENDS[:-1]
DMA_CHUNKS = len(CHUNK_ENDS)
ACOLS = NSTEP * FREE  # columns in each activation stream

F32 = mybir.dt.float32
BF16 = mybir.dt.bfloat16
BF16NP = ml_dtypes.bfloat16

SHARED_INPUTS = ["r2pack", "r4pack"]

# bfin column layout: init-critical columns first (they load in a separate
# earlier DMA so the state-init matmuls can start before the Mz tables land)
C_IB16 = 0
C_ZHI = 128
C_ZLO = C_ZHI + FREE
C_YHI = C_ZLO + FREE
C_YLO = C_YHI + FREE
C_BV = C_YLO + FREE
C_CRIT = C_BV + FREE
C_MZT = C_CRIT
C_MZTL = C_MZT + NMZ
C_MZTE = C_MZTL + NMZ
C_TOT = C_MZTE + NMZ


def _coefficients():
    """Exact fp64 scalar recursions for the output-extraction weights."""
    gamma = np.zeros(NEVAL)
    la = np.zeros(NEVAL)
    alpha_y = alpha_z = 1.0
    nu_y = nu_z = 0.0
    for s in range(NSTEP):
        la[2 * s] += -HSTEP
        nu_z += -HSTEP
        gamma *= INVL
        alpha_y *= INVL
        nu_y *= INVL
        gamma += (1.0 - INVL) * la
        alpha_y += (1.0 - INVL) * alpha_z
        nu_y += (1.0 - INVL) * nu_z
        gamma[2 * s + 1] += -INVL * HSTEP
        nu_y += -INVL * HSTEP
    return gamma, alpha_y, nu_y


def _pack_mz(M):
    """pack[p, (k*NBLK+j)*128 + m] = M[128*j+m, 128*k+p]"""
    MT = M.T
    pack = np.zeros((128, NMZ))
    for k in range(NBLK):
        for j in range(NBLK):
            pack[:, (k * NBLK + j) * 128 : (k * NBLK + j + 1) * 128] = MT[
                128 * k : 128 * k + 128, 128 * j : 128 * j + 128
            ]
    return pack


def _host_tables(W1, b1, u1, W2, b2):
    """All precomputed tensors, fp64 internally."""
    W1 = W1.astype(np.float64)
    W2 = W2.astype(np.float64)
    b1 = b1.astype(np.float64)
    u1 = u1.astype(np.float64)
    b2 = b2.astype(np.float64)

    Mz = -HSTEP * (W1 @ W2)  # [H, H]
    W1b2 = W1 @ b2  # [H]
    l = LCOUP

    def be(s):
        return b1 + (1.0 - s * HSTEP) * u1

    def bp(s):  # beta_odd
        return b1 + (1.0 - (s + 1) * HSTEP) * u1 - (s + 1) * HSTEP * W1b2

    # rank-2 bias tables: lhsT slice [2, 128] at cols 128*s
    dbz = np.zeros((2, NSTEP * 128))
    dby = np.zeros((2, NSTEP * 128))
    dbys = np.zeros((NSTEP, H))
    for s in range(NSTEP):
        dz = bp(s) if s == 0 else bp(s) - bp(s - 1)
        for k in range(NBLK):
            dbz[k, s * 128 : (s + 1) * 128] = dz[128 * k : 128 * k + 128]
    for s in range(NSTEP - 1):
        dh = -HSTEP * W1b2 + l * be(s + 1) - (l - 1.0) * bp(s) - be(s)
        dbys[s] = dh
        for k in range(NBLK):
            dby[k, s * 128 : (s + 1) * 128] = dh[128 * k : 128 * k + 128]

    # rank-4 B-bias table: rows 0-1 = eps*dby(s), rows 2-3 = (l-1)*dbz(s+1)
    dbv = np.zeros((4, NSTEP * 128))
    for s in range(NSTEP - 1):
        ev = EPS * dbys[s]
        lz = (l - 1.0) * (bp(s + 1) - bp(s))
        for k in range(NBLK):
            dbv[k, s * 128 : (s + 1) * 128] = ev[128 * k : 128 * k + 128]
            dbv[2 + k, s * 128 : (s + 1) * 128] = lz[128 * k : 128 * k + 128]

    ind = np.zeros((2, FREE))
    for k in range(NBLK):
        ind[k, k * BSH : (k + 1) * BSH] = 1.0
    indv = np.concatenate([ind, ind], axis=0)  # [4, FREE]

    mzpack = np.concatenate(
        [_pack_mz(Mz), _pack_mz((l - 1.0) * Mz), _pack_mz(EPS * Mz)], axis=1
    ).astype(BF16NP)  # [128, 3*NMZ]
    ib16 = np.eye(128).astype(BF16NP)

    r2pack = np.concatenate([dbz, dby, ind], axis=1).astype(BF16NP)
    r4pack = np.concatenate([dbv, indv], axis=1).astype(BF16NP)

    return dict(mzpack=mzpack, ib16=ib16, r2pack=r2pack, r4pack=r4pack)


def _build_kernel():
    """Build the Bass module (same program for every core)."""
    nc = bacc.Bacc("TRN2", target_bir_lowering=False, debug=False)

    bfin_d = nc.dram_tensor("bfin", [128, C_TOT], BF16, kind="ExternalInput").ap()
    r2pack_d = nc.dram_tensor("r2pack", [2, 2 * NSTEP * 128 + FREE], BF16, kind="ExternalInput").ap()
    r4pack_d = nc.dram_tensor("r4pack", [4, NSTEP * 128 + FREE], BF16, kind="ExternalInput").ap()

    a_out_d = nc.dram_tensor("a_out0", [128, 2 * ACOLS], BF16, kind="ExternalOutput").ap()

    with tile.TileContext(nc) as tc, ExitStack() as ctx:
        consts = ctx.enter_context(tc.tile_pool(name="consts", bufs=1))
        zpool = ctx.enter_context(tc.tile_pool(name="zps", bufs=1, space="PSUM"))
        ypool = ctx.enter_context(tc.tile_pool(name="yps", bufs=1, space="PSUM"))
        bpool = ctx.enter_context(tc.tile_pool(name="bps", bufs=1, space="PSUM"))
        ppool = ctx.enter_context(tc.tile_pool(name="ptmp", bufs=2))

        # --- prime the tanh activation table early (dep-free) ---
        warm = consts.tile([1, 8], F32, tag="warm")
        nc.vector.memset(warm[:], 0.0)
        nc.scalar.activation(warm[:], warm[:], mybir.ActivationFunctionType.Tanh)

        # --- load constants (4 DMAs; the small init-critical slice first) ---
        bfin = consts.tile([128, C_TOT], BF16, tag="bfin", name="bfin")
        nc.sync.dma_start(bfin[:, 0:C_CRIT], bfin_d[:, 0:C_CRIT])
        nc.sync.dma_start(bfin[:, C_CRIT:C_TOT], bfin_d[:, C_CRIT:C_TOT])
        mzt = bfin[:, C_MZT : C_MZT + NMZ]
        mztl = bfin[:, C_MZTL : C_MZTL + NMZ]
        mzte = bfin[:, C_MZTE : C_MZTE + NMZ]
        ib16 = bfin[:, C_IB16 : C_IB16 + 128]

        r2pack = consts.tile([2, 2 * NSTEP * 128 + FREE], BF16, tag="r2pack", name="r2pack")
        nc.sync.dma_start(r2pack[:], r2pack_d)
        dbz = r2pack[:, 0 : NSTEP * 128]
        dby = r2pack[:, NSTEP * 128 : 2 * NSTEP * 128]
        indb = r2pack[:, 2 * NSTEP * 128 : 2 * NSTEP * 128 + FREE]

        r4pack = consts.tile([4, NSTEP * 128 + FREE], BF16, tag="r4pack", name="r4pack")
        nc.sync.dma_start(r4pack[:], r4pack_d)
        dbv = r4pack[:, 0 : NSTEP * 128]
        indv = r4pack[:, NSTEP * 128 : NSTEP * 128 + FREE]

        # --- init states via bf16 hi/lo identity matmuls ---
        z_ps = zpool.tile([128, FREE], F32, tag="z", name="z")
        y_ps = ypool.tile([128, FREE], F32, tag="y", name="y")
        b_ps = bpool.tile([128, FREE], F32, tag="b", name="b")
        nc.tensor.matmul(z_ps[:], ib16, bfin[:, C_ZHI : C_ZHI + FREE], start=True, stop=False)
        nc.tensor.matmul(z_ps[:], ib16, bfin[:, C_ZLO : C_ZLO + FREE], start=False, stop=True)
        nc.tensor.matmul(y_ps[:], ib16, bfin[:, C_YHI : C_YHI + FREE], start=True, stop=False)
        nc.tensor.matmul(y_ps[:], ib16, bfin[:, C_YLO : C_YLO + FREE], start=False, stop=True)
        nc.tensor.matmul(b_ps[:], ib16, bfin[:, C_BV : C_BV + FREE], start=True, stop=True)

        abuf = [
            consts.tile([128, 2 * (e - s0) * FREE], BF16, tag=f"ab{c}", name=f"ab{c}")
            for c, (s0, e) in enumerate(zip(CHUNK_STARTS, CHUNK_ENDS))
        ]

        def blk(tab, k, j):
            base = (k * NBLK + j) * 128
            return tab[:, base : base + 128]

        for s in range(NSTEP):
            last = s == NSTEP - 1
            chunk = next(c for c, e in enumerate(CHUNK_ENDS) if s < e)
            ecol = (s - CHUNK_STARTS[chunk]) * FREE

            if s > 0:
                # z bias delta for THIS step (WAR on last step's odd ACT)
                nc.tensor.matmul(
                    z_ps[:], dbz[:, s * 128 : (s + 1) * 128], indb[:],
                    start=False, stop=False, skip_group_check=True,
                )

            if not last:
                # the ONLY vector op: b = bf16(B) (B complete since the
                # previous step's tail; B has no other readers)
                b_t = ppool.tile([128, FREE], BF16, tag="b", name=f"b_{s}")
                nc.vector.tensor_copy(b_t[:], b_ps[:])

            half = (CHUNK_ENDS[chunk] - CHUNK_STARTS[chunk]) * FREE

            # --- even eval: a_e = tanh(inv_l * Yhat) ---
            a_even = abuf[chunk][:, ecol : ecol + FREE]
            nc.scalar.activation(
                a_even[:], y_ps[:], mybir.ActivationFunctionType.Tanh,
                scale=INVL,
            )

            # --- Z += Mz @ a_even ---
            for j in range(NBLK):
                for k in range(NBLK):
                    nc.tensor.matmul(
                        z_ps[:, j * BSH : (j + 1) * BSH],
                        blk(mzt, k, j),
                        a_even[:, k * BSH : (k + 1) * BSH],
                        start=False,
                        stop=False,
                        skip_group_check=True,
                    )

            # --- odd eval: a_o = tanh(Z) --- (emitted before the Y/B matmul
            # pack so its tensor-side wait covers only the z matmuls)
            a_odd = abuf[chunk][:, half + ecol : half + ecol + FREE]
            nc.scalar.activation(
                a_odd[:], z_ps[:], mybir.ActivationFunctionType.Tanh, scale=1.0
            )

            if not last:
                # --- Y += dby + b + (l-1)Mz a_e + Mz a_o ---
                nc.tensor.matmul(
                    y_ps[:], dby[:, s * 128 : (s + 1) * 128], indb[:],
                    start=False, stop=False, skip_group_check=True,
                )
                nc.tensor.matmul(
                    y_ps[:], ib16, b_t[:],
                    start=False, stop=False, skip_group_check=True,
                )
                for j in range(NBLK):
                    for k in range(NBLK):
                        nc.tensor.matmul(
                            y_ps[:, j * BSH : (j + 1) * BSH],
                            blk(mztl, k, j),
                            a_even[:, k * BSH : (k + 1) * BSH],
                            start=False,
                            stop=False,
                            skip_group_check=True,
                        )
                for j in range(NBLK):
                    for k in range(NBLK):
                        nc.tensor.matmul(
                            y_ps[:, j * BSH : (j + 1) * BSH],
                            blk(mzt, k, j),
                            a_odd[:, k * BSH : (k + 1) * BSH],
                            start=False,
                            stop=False,
                            skip_group_check=True,
                        )

                # --- B rebuild: inv_l b + inv_l(l-1)Mz a_e + eps Mz a_o
                #     + rank-4 bias (inv_l absorbed: ~1e-6 relative) ---
                nc.tensor.matmul(b_ps[:], ib16, b_t[:], start=True, stop=False)
                for j in range(NBLK):
                    for k in range(NBLK):
                        nc.tensor.matmul(
                            b_ps[:, j * BSH : (j + 1) * BSH],
                            blk(mztl, k, j),
                            a_even[:, k * BSH : (k + 1) * BSH],
                            start=False,
                            stop=False,
                            skip_group_check=True,
                        )
                for j in range(NBLK):
                    for k in range(NBLK):
                        nc.tensor.matmul(
                            b_ps[:, j * BSH : (j + 1) * BSH],
                            blk(mzte, k, j),
                            a_odd[:, k * BSH : (k + 1) * BSH],
                            start=False,
                            stop=False,
                            skip_group_check=True,
                        )
                nc.tensor.matmul(
                    b_ps[:], dbv[:, s * 128 : (s + 1) * 128], indv[:],
                    start=False, stop=True, skip_group_check=True,
                )

            if s + 1 == CHUNK_ENDS[chunk]:
                c0 = 2 * CHUNK_STARTS[chunk] * FREE
                nc.sync.dma_start(a_out_d[:, c0 : c0 + 2 * half], abuf[chunk][:])

    nc.compile()
    return nc


_CACHE = {}


def _get_kernel():
    if "nc" not in _CACHE:
        _CACHE["nc"] = _build_kernel()
    return _CACHE["nc"]


def kernel(y1, W1, b1, u1, W2, b2, _trace=False, _trace_kwargs=None):
    y1 = np.asarray(y1)
    in_dtype = y1.dtype
    W1_ = np.asarray(W1, dtype=np.float64)
    W2_ = np.asarray(W2, dtype=np.float64)
    b2_ = np.asarray(b2, dtype=np.float64)
    tabs = _host_tables(
        np.asarray(W1), np.asarray(b1), np.asarray(u1), np.asarray(W2), np.asarray(b2)
    )

    nc = _get_kernel()

    # exact fp64 host init: Z = W1 y1 + bp(0), Yhat = l (W1 y1 + be(0)),
    # B = eps*Yhat + (l-1)*Z, packed in the (block, sample) column layout
    b1_ = np.asarray(b1, dtype=np.float64)
    u1_ = np.asarray(u1, dtype=np.float64)
    W1b2 = W1_ @ b2_
    be0 = b1_ + u1_
    bp0 = b1_ + (1.0 - HSTEP) * u1_ - HSTEP * W1b2

    shared = {k: tabs[k] for k in SHARED_INPUTS}
    in_maps = []
    for c in range(NCORES):
        m = dict(shared)
        r0 = c * BS
        shard = y1[r0 : r0 + BSH].astype(np.float64)  # [BSH, D]
        wy = W1_ @ shard.T  # [H, BSH]
        zi = wy + bp0[:, None]
        yi = LCOUP * (wy + be0[:, None])
        bi = EPS * yi + (LCOUP - 1.0) * zi
        packz = np.zeros((128, FREE), dtype=np.float32)
        packy = np.zeros((128, FREE), dtype=np.float32)
        packb = np.zeros((128, FREE), dtype=np.float32)
        for j in range(NBLK):
            packz[:, j * BSH : (j + 1) * BSH] = zi[128 * j : 128 * (j + 1)]
            packy[:, j * BSH : (j + 1) * BSH] = yi[128 * j : 128 * (j + 1)]
            packb[:, j * BSH : (j + 1) * BSH] = bi[128 * j : 128 * (j + 1)]
        zhi = packz.astype(BF16NP)
        zlo = (packz - zhi.astype(np.float32)).astype(BF16NP)
        yhi = packy.astype(BF16NP)
        ylo = (packy - yhi.astype(np.float32)).astype(BF16NP)
        bv = packb.astype(BF16NP)
        m["bfin"] = np.concatenate(
            [tabs["ib16"], zhi, zlo, yhi, ylo, bv, tabs["mzpack"]], axis=1
        )
        in_maps.append(m)

    kw = {}
    if _trace:
        kw["trace"] = True
        if _trace_kwargs:
            kw.update(_trace_kwargs)
    res = run_bass_kernel_spmd(nc, in_maps, core_ids=list(range(NCORES)), **kw)

    # --- exact host-side output extraction ---
    gamma, c_y, c_b = _coefficients()
    cvec = np.sum(W1_ * W2_.T, axis=1)  # diag(W1@W2)
    sum_c = float(np.sum(cvec))

    out = np.zeros((B, D + 1), dtype=np.float32)
    for c in range(NCORES):
        araw = np.asarray(res.results[c]["a_out0"]).astype(np.float64)
        ae = np.zeros((128, NSTEP, NBLK, BSH))  # [p, s, blk, b]
        ao = np.zeros((128, NSTEP, NBLK, BSH))
        for ci, (s0, e) in enumerate(zip(CHUNK_STARTS, CHUNK_ENDS)):
            c0 = 2 * s0 * FREE
            half = (e - s0) * FREE
            ae[:, s0:e] = araw[:, c0 : c0 + half].reshape(128, e - s0, NBLK, BSH)
            ao[:, s0:e] = araw[:, c0 + half : c0 + 2 * half].reshape(
                128, e - s0, NBLK, BSH
            )
        ae = np.moveaxis(ae, (2, 0), (1, 2)).reshape(NSTEP, H, BSH)  # [s,h,b]
        ao = np.moveaxis(ao, (2, 0), (1, 2)).reshape(NSTEP, H, BSH)

        S = np.einsum("s,shb->hb", gamma[0::2], ae) + np.einsum(
            "s,shb->hb", gamma[1::2], ao
        )
        r0 = c * BS
        shard = y1[r0 : r0 + BSH].astype(np.float64)  # [BSH, D]
        y_fin = c_y * shard + (W2_ @ S).T + c_b * b2_[None, :]
        ptr = np.einsum("h,shb->b", cvec, ae**2)
        i_fin = HSTEP * (NSTEP * sum_c - ptr)
        out[r0 : r0 + BSH, :D] = y_fin.astype(np.float32)
        out[r0 : r0 + BSH, D] = i_fin.astype(np.float32)

    if _trace:
        return out.astype(in_dtype, copy=False), res
    return out.astype(in_dtype, copy=False)


# revision 15
# speedup vs baseline: 1.0067x; 1.0067x over previous
"""Trainium2 Bass kernel for the CNF reversible backward solve.

Math restructuring (exact, validated in fp64 against the jax reference):

The per-step recursion is tracked purely in H-space (H=256) via
Z = W1 z + bp(s), Yhat = l*(W1 y + be(s)):
    a_even = tanh(inv_l * Yhat)
    Z     += Mz @ a_even  (+ per-step bias delta)      (Mz = -h W1 W2)
    a_odd  = tanh(Z)
    Yhat' += dby + [eps*Yhat + (l-1)*Z_post] + Mz a_odd,  eps = inv_l - 1

All three live states are PSUM accumulations updated by matmuls only:
Z, Yhat, and a third bank B(s) = eps*Yhat_s + (l-1)*(Z_s + dbz_s) that
carries the ~1e-3-scaled coupling correction.  B obeys the pure-matmul
recursion
    B(s+1) = inv_l*B(s) + inv_l(l-1)*Mz a_e + eps*Mz a_o + rank-4 bias
so the correction enters Y as [one bf16 identity matmul of b=bf16(B)] +
[(l-1)Mz @ a_e block matmuls], and B is rebuilt from the same b with
scaled-Mz tables.  The ONLY vector-engine op per step is the bf16 copy
b = bf16(B) at step start; each PSUM bank has exactly ONE reader (Y:
even ACT, Z: odd ACT, B: the copy), which matters because the tile
framework chains same-tile readers across engines in emission order.
The large states never leave fp32 PSUM; everything bf16-routed is
1e-3-scaled, so rounding is negligible (measured 3.2e-4 end to end).

The device streams all activations a_e, a_o to DRAM; the D-space outputs
are exact fp64 host-side postprocessing:
    y_final = c_y y1 + sum_e gamma_e (W2 @ a_e) + c_b b2
    I_final = h (N sum(c) - sum_s c . a_even_s^2),   c = diag(W1 W2)

Sharding: data-parallel, B=256 -> 32 samples on each of 8 cores;
parameters replicated; gather + assembly on host.
"""

import numpy as np
import ml_dtypes
from contextlib import ExitStack

import concourse.bass as bass
import concourse.tile as tile
from concourse import bacc, mybir
from concourse.bass_utils import run_bass_kernel_spmd

# Problem constants (hardcoded per contract)
NCORES = 8
B, D, H = 256, 64, 256
NSTEP = 64
HSTEP = 1.0 / NSTEP
LCOUP = 0.999
INVL = 1.0 / LCOUP
EPS = INVL - 1.0
BS = B // NCORES  # 32 samples per core
BSH = BS
NBLK = H // 128  # 2 h-blocks
FREE = NBLK * BSH  # 64: free size of H-space tiles, layout (blk, sample)
NEVAL = 2 * NSTEP  # 128
NMZ = NBLK * NBLK * 128  # 512 columns per packed Mz table

# uneven out-DMA chunks: small final chunk shortens the post-loop tail
CHUNK_ENDS = [16, 32, 48, 62, 64]
CHUNK_STARTS = [0] + CHUNK_ENDS[:-1]
DMA_CHUNKS = len(CHUNK_ENDS)
ACOLS = NSTEP * FREE  # columns in each activation stream

F32 = mybir.dt.float32
BF16 = mybir.dt.bfloat16
BF16NP = ml_dtypes.bfloat16

SHARED_INPUTS = ["r2pack", "r4pack"]

# bfin column layout: init-critical columns first (they load in a separate
# earlier DMA so the state-init matmuls can start before the Mz tables land)
C_IB16 = 0
C_ZHI = 128
C_ZLO = C_ZHI + FREE
C_YHI = C_ZLO + FREE
C_YLO = C_YHI + FREE
C_BV = C_YLO + FREE
C_MZT = C_BV + FREE
C_CRIT = C_MZT + NMZ  # first DMA: ib16 + init states + mzt
C_MZTL = C_CRIT
C_MZTE = C_MZTL + NMZ
C_TOT = C_MZTE + NMZ


def _coefficients():
    """Exact fp64 scalar recursions for the output-extraction weights."""
    gamma = np.zeros(NEVAL)
    la = np.zeros(NEVAL)
    alpha_y = alpha_z = 1.0
    nu_y = nu_z = 0.0
    for s in range(NSTEP):
        la[2 * s] += -HSTEP
        nu_z += -HSTEP
        gamma *= INVL
        alpha_y *= INVL
        nu_y *= INVL
        gamma += (1.0 - INVL) * la
        alpha_y += (1.0 - INVL) * alpha_z
        nu_y += (1.0 - INVL) * nu_z
        gamma[2 * s + 1] += -INVL * HSTEP
        nu_y += -INVL * HSTEP
    return gamma, alpha_y, nu_y


def _pack_mz(M):
    """pack[p, (k*NBLK+j)*128 + m] = M[128*j+m, 128*k+p]"""
    MT = M.T
    pack = np.zeros((128, NMZ))
    for k in range(NBLK):
        for j in range(NBLK):
            pack[:, (k * NBLK + j) * 128 : (k * NBLK + j + 1) * 128] = MT[
                128 * k : 128 * k + 128, 128 * j : 128 * j + 128
            ]
    return pack


def _host_tables(W1, b1, u1, W2, b2):
    """All precomputed tensors, fp64 internally."""
    W1 = W1.astype(np.float64)
    W2 = W2.astype(np.float64)
    b1 = b1.astype(np.float64)
    u1 = u1.astype(np.float64)
    b2 = b2.astype(np.float64)

    Mz = -HSTEP * (W1 @ W2)  # [H, H]
    W1b2 = W1 @ b2  # [H]
    l = LCOUP

    def be(s):
        return b1 + (1.0 - s * HSTEP) * u1

    def bp(s):  # beta_odd
        return b1 + (1.0 - (s + 1) * HSTEP) * u1 - (s + 1) * HSTEP * W1b2

    # rank-2 bias tables: lhsT slice [2, 128] at cols 128*s
    dbz = np.zeros((2, NSTEP * 128))
    dby = np.zeros((2, NSTEP * 128))
    dbys = np.zeros((NSTEP, H))
    for s in range(NSTEP):
        dz = bp(s) if s == 0 else bp(s) - bp(s - 1)
        for k in range(NBLK):
            dbz[k, s * 128 : (s + 1) * 128] = dz[128 * k : 128 * k + 128]
    for s in range(NSTEP - 1):
        dh = -HSTEP * W1b2 + l * be(s + 1) - (l - 1.0) * bp(s) - be(s)
        dbys[s] = dh
        for k in range(NBLK):
            dby[k, s * 128 : (s + 1) * 128] = dh[128 * k : 128 * k + 128]

    # rank-4 B-bias table: rows 0-1 = eps*dby(s), rows 2-3 = (l-1)*dbz(s+1)
    dbv = np.zeros((4, NSTEP * 128))
    for s in range(NSTEP - 1):
        ev = EPS * dbys[s]
        lz = (l - 1.0) * (bp(s + 1) - bp(s))
        for k in range(NBLK):
            dbv[k, s * 128 : (s + 1) * 128] = ev[128 * k : 128 * k + 128]
            dbv[2 + k, s * 128 : (s + 1) * 128] = lz[128 * k : 128 * k + 128]

    ind = np.zeros((2, FREE))
    for k in range(NBLK):
        ind[k, k * BSH : (k + 1) * BSH] = 1.0
    indv = np.concatenate([ind, ind], axis=0)  # [4, FREE]

    mzpack = np.concatenate(
        [_pack_mz(Mz), _pack_mz((l - 1.0) * Mz), _pack_mz(EPS * Mz)], axis=1
    ).astype(BF16NP)  # [128, 3*NMZ]
    ib16 = np.eye(128).astype(BF16NP)

    r2pack = np.concatenate([dbz, dby, ind], axis=1).astype(BF16NP)
    r4pack = np.concatenate([dbv, indv], axis=1).astype(BF16NP)

    return dict(mzpack=mzpack, ib16=ib16, r2pack=r2pack, r4pack=r4pack)


def _build_kernel():
    """Build the Bass module (same program for every core)."""
    nc = bacc.Bacc("TRN2", target_bir_lowering=False, debug=False)

    bfin_d = nc.dram_tensor("bfin", [128, C_TOT], BF16, kind="ExternalInput").ap()
    r2pack_d = nc.dram_tensor("r2pack", [2, 2 * NSTEP * 128 + FREE], BF16, kind="ExternalInput").ap()
    r4pack_d = nc.dram_tensor("r4pack", [4, NSTEP * 128 + FREE], BF16, kind="ExternalInput").ap()

    a_out_d = nc.dram_tensor("a_out0", [128, 2 * ACOLS], BF16, kind="ExternalOutput").ap()

    with tile.TileContext(nc) as tc, ExitStack() as ctx:
        consts = ctx.enter_context(tc.tile_pool(name="consts", bufs=1))
        zpool = ctx.enter_context(tc.tile_pool(name="zps", bufs=1, space="PSUM"))
        ypool = ctx.enter_context(tc.tile_pool(name="yps", bufs=1, space="PSUM"))
        bpool = ctx.enter_context(tc.tile_pool(name="bps", bufs=1, space="PSUM"))
        ppool = ctx.enter_context(tc.tile_pool(name="ptmp", bufs=2))

        # --- prime the tanh activation table early (dep-free; input is
        # uninitialized SBUF, which is fine for a table prime) ---
        warm = consts.tile([1, 8], F32, tag="warm")
        nc.scalar.activation(warm[:], warm[:], mybir.ActivationFunctionType.Tanh)

        # --- load constants; input DMAs issue from the vector/gpsimd
        # queues whose preambles finish earlier than Sync's (Sync keeps
        # the output DMAs) ---
        bfin = consts.tile([128, C_TOT], BF16, tag="bfin", name="bfin")
        nc.sync.dma_start(bfin[:, 0:C_CRIT], bfin_d[:, 0:C_CRIT])
        nc.sync.dma_start(bfin[:, C_CRIT:C_TOT], bfin_d[:, C_CRIT:C_TOT])
        mzt = bfin[:, C_MZT : C_MZT + NMZ]
        mztl = bfin[:, C_MZTL : C_MZTL + NMZ]
        mzte = bfin[:, C_MZTE : C_MZTE + NMZ]
        ib16 = bfin[:, C_IB16 : C_IB16 + 128]

        r2pack = consts.tile([2, 2 * NSTEP * 128 + FREE], BF16, tag="r2pack", name="r2pack")
        nc.gpsimd.dma_start(r2pack[:], r2pack_d)
        dbz = r2pack[:, 0 : NSTEP * 128]
        dby = r2pack[:, NSTEP * 128 : 2 * NSTEP * 128]
        indb = r2pack[:, 2 * NSTEP * 128 : 2 * NSTEP * 128 + FREE]

        r4pack = consts.tile([4, NSTEP * 128 + FREE], BF16, tag="r4pack", name="r4pack")
        nc.gpsimd.dma_start(r4pack[:], r4pack_d)
        dbv = r4pack[:, 0 : NSTEP * 128]
        indv = r4pack[:, NSTEP * 128 : NSTEP * 128 + FREE]

        # --- init states via bf16 hi/lo identity matmuls ---
        z_ps = zpool.tile([128, FREE], F32, tag="z", name="z")
        y_ps = ypool.tile([128, FREE], F32, tag="y", name="y")
        b_ps = bpool.tile([128, FREE], F32, tag="b", name="b")
        nc.tensor.matmul(z_ps[:], ib16, bfin[:, C_ZHI : C_ZHI + FREE], start=True, stop=False)
        nc.tensor.matmul(z_ps[:], ib16, bfin[:, C_ZLO : C_ZLO + FREE], start=False, stop=True)
        nc.tensor.matmul(y_ps[:], ib16, bfin[:, C_YHI : C_YHI + FREE], start=True, stop=False)
        nc.tensor.matmul(y_ps[:], ib16, bfin[:, C_YLO : C_YLO + FREE], start=False, stop=True)
        nc.tensor.matmul(b_ps[:], ib16, bfin[:, C_BV : C_BV + FREE], start=True, stop=True)

        abuf = [
            consts.tile([128, 2 * (e - s0) * FREE], BF16, tag=f"ab{c}", name=f"ab{c}")
            for c, (s0, e) in enumerate(zip(CHUNK_STARTS, CHUNK_ENDS))
        ]

        def blk(tab, k, j):
            base = (k * NBLK + j) * 128
            return tab[:, base : base + 128]

        for s in range(NSTEP):
            last = s == NSTEP - 1
            chunk = next(c for c, e in enumerate(CHUNK_ENDS) if s < e)
            ecol = (s - CHUNK_STARTS[chunk]) * FREE

            if s > 0:
                # z bias delta for THIS step (WAR on last step's odd ACT)
                nc.tensor.matmul(
                    z_ps[:], dbz[:, s * 128 : (s + 1) * 128], indb[:],
                    start=False, stop=False, skip_group_check=True,
                )

            if not last:
                # the ONLY vector op: b = bf16(B) (B complete since the
                # previous step's tail; B has no other readers)
                b_t = ppool.tile([128, FREE], BF16, tag="b", name=f"b_{s}")
                nc.vector.tensor_copy(b_t[:], b_ps[:])

            half = (CHUNK_ENDS[chunk] - CHUNK_STARTS[chunk]) * FREE

            # --- even eval: a_e = tanh(inv_l * Yhat) ---
            a_even = abuf[chunk][:, ecol : ecol + FREE]
            nc.scalar.activation(
                a_even[:], y_ps[:], mybir.ActivationFunctionType.Tanh,
                scale=INVL,
            )

            # --- Z += Mz @ a_even ---
            for j in range(NBLK):
                for k in range(NBLK):
                    nc.tensor.matmul(
                        z_ps[:, j * BSH : (j + 1) * BSH],
                        blk(mzt, k, j),
                        a_even[:, k * BSH : (k + 1) * BSH],
                        start=False,
                        stop=False,
                        skip_group_check=True,
                    )

            # --- odd eval: a_o = tanh(Z) --- (emitted before the Y/B matmul
            # pack so its tensor-side wait covers only the z matmuls)
            a_odd = abuf[chunk][:, half + ecol : half + ecol + FREE]
            nc.scalar.activation(
                a_odd[:], z_ps[:], mybir.ActivationFunctionType.Tanh, scale=1.0
            )

            if not last:
                # --- Y += dby + b + (l-1)Mz a_e + Mz a_o ---
                nc.tensor.matmul(
                    y_ps[:], dby[:, s * 128 : (s + 1) * 128], indb[:],
                    start=False, stop=False, skip_group_check=True,
                )
                nc.tensor.matmul(
                    y_ps[:], ib16, b_t[:],
                    start=False, stop=False, skip_group_check=True,
                )
                for j in range(NBLK):
                    for k in range(NBLK):
                        nc.tensor.matmul(
                            y_ps[:, j * BSH : (j + 1) * BSH],
                            blk(mztl, k, j),
                            a_even[:, k * BSH : (k + 1) * BSH],
                            start=False,
                            stop=False,
                            skip_group_check=True,
                        )
                for j in range(NBLK):
                    for k in range(NBLK):
                        nc.tensor.matmul(
                            y_ps[:, j * BSH : (j + 1) * BSH],
                            blk(mzt, k, j),
                            a_odd[:, k * BSH : (k + 1) * BSH],
                            start=False,
                            stop=False,
                            skip_group_check=True,
                        )

                # --- B rebuild: inv_l b + inv_l(l-1)Mz a_e + eps Mz a_o
                #     + rank-4 bias (inv_l absorbed: ~1e-6 relative) ---
                nc.tensor.matmul(b_ps[:], ib16, b_t[:], start=True, stop=False)
                for j in range(NBLK):
                    for k in range(NBLK):
                        nc.tensor.matmul(
                            b_ps[:, j * BSH : (j + 1) * BSH],
                            blk(mztl, k, j),
                            a_even[:, k * BSH : (k + 1) * BSH],
                            start=False,
                            stop=False,
                            skip_group_check=True,
                        )
                for j in range(NBLK):
                    for k in range(NBLK):
                        nc.tensor.matmul(
                            b_ps[:, j * BSH : (j + 1) * BSH],
                            blk(mzte, k, j),
                            a_odd[:, k * BSH : (k + 1) * BSH],
                            start=False,
                            stop=False,
                            skip_group_check=True,
                        )
                nc.tensor.matmul(
                    b_ps[:], dbv[:, s * 128 : (s + 1) * 128], indv[:],
                    start=False, stop=True, skip_group_check=True,
                )

            if s + 1 == CHUNK_ENDS[chunk]:
                c0 = 2 * CHUNK_STARTS[chunk] * FREE
                nc.sync.dma_start(a_out_d[:, c0 : c0 + 2 * half], abuf[chunk][:])

    nc.compile()
    return nc


_CACHE = {}


def _get_kernel():
    if "nc" not in _CACHE:
        _CACHE["nc"] = _build_kernel()
    return _CACHE["nc"]


def kernel(y1, W1, b1, u1, W2, b2, _trace=False, _trace_kwargs=None):
    y1 = np.asarray(y1)
    in_dtype = y1.dtype
    W1_ = np.asarray(W1, dtype=np.float64)
    W2_ = np.asarray(W2, dtype=np.float64)
    b2_ = np.asarray(b2, dtype=np.float64)
    tabs = _host_tables(
        np.asarray(W1), np.asarray(b1), np.asarray(u1), np.asarray(W2), np.asarray(b2)
    )

    nc = _get_kernel()

    # exact fp64 host init: Z = W1 y1 + bp(0), Yhat = l (W1 y1 + be(0)),
    # B = eps*Yhat + (l-1)*Z, packed in the (block, sample) column layout
    b1_ = np.asarray(b1, dtype=np.float64)
    u1_ = np.asarray(u1, dtype=np.float64)
    W1b2 = W1_ @ b2_
    be0 = b1_ + u1_
    bp0 = b1_ + (1.0 - HSTEP) * u1_ - HSTEP * W1b2

    shared = {k: tabs[k] for k in SHARED_INPUTS}
    in_maps = []
    for c in range(NCORES):
        m = dict(shared)
        r0 = c * BS
        shard = y1[r0 : r0 + BSH].astype(np.float64)  # [BSH, D]
        wy = W1_ @ shard.T  # [H, BSH]
        zi = wy + bp0[:, None]
        yi = LCOUP * (wy + be0[:, None])
        bi = EPS * yi + (LCOUP - 1.0) * zi
        packz = np.zeros((128, FREE), dtype=np.float32)
        packy = np.zeros((128, FREE), dtype=np.float32)
        packb = np.zeros((128, FREE), dtype=np.float32)
        for j in range(NBLK):
            packz[:, j * BSH : (j + 1) * BSH] = zi[128 * j : 128 * (j + 1)]
            packy[:, j * BSH : (j + 1) * BSH] = yi[128 * j : 128 * (j + 1)]
            packb[:, j * BSH : (j + 1) * BSH] = bi[128 * j : 128 * (j + 1)]
        zhi = packz.astype(BF16NP)
        zlo = (packz - zhi.astype(np.float32)).astype(BF16NP)
        yhi = packy.astype(BF16NP)
        ylo = (packy - yhi.astype(np.float32)).astype(BF16NP)
        bv = packb.astype(BF16NP)
        m["bfin"] = np.concatenate(
            [tabs["ib16"], zhi, zlo, yhi, ylo, bv, tabs["mzpack"]], axis=1
        )  # mzpack order: mzt | mztl | mzte matches C_MZT < C_MZTL < C_MZTE
        in_maps.append(m)

    kw = {}
    if _trace:
        kw["trace"] = True
        if _trace_kwargs:
            kw.update(_trace_kwargs)
    res = run_bass_kernel_spmd(nc, in_maps, core_ids=list(range(NCORES)), **kw)

    # --- exact host-side output extraction ---
    gamma, c_y, c_b = _coefficients()
    cvec = np.sum(W1_ * W2_.T, axis=1)  # diag(W1@W2)
    sum_c = float(np.sum(cvec))

    out = np.zeros((B, D + 1), dtype=np.float32)
    for c in range(NCORES):
        araw = np.asarray(res.results[c]["a_out0"]).astype(np.float64)
        ae = np.zeros((128, NSTEP, NBLK, BSH))  # [p, s, blk, b]
        ao = np.zeros((128, NSTEP, NBLK, BSH))
        for ci, (s0, e) in enumerate(zip(CHUNK_STARTS, CHUNK_ENDS)):
            c0 = 2 * s0 * FREE
            half = (e - s0) * FREE
            ae[:, s0:e] = araw[:, c0 : c0 + half].reshape(128, e - s0, NBLK, BSH)
            ao[:, s0:e] = araw[:, c0 + half : c0 + 2 * half].reshape(
                128, e - s0, NBLK, BSH
            )
        ae = np.moveaxis(ae, (2, 0), (1, 2)).reshape(NSTEP, H, BSH)  # [s,h,b]
        ao = np.moveaxis(ao, (2, 0), (1, 2)).reshape(NSTEP, H, BSH)

        S = np.einsum("s,shb->hb", gamma[0::2], ae) + np.einsum(
            "s,shb->hb", gamma[1::2], ao
        )
        r0 = c * BS
        shard = y1[r0 : r0 + BSH].astype(np.float64)  # [BSH, D]
        y_fin = c_y * shard + (W2_ @ S).T + c_b * b2_[None, :]
        ptr = np.einsum("h,shb->b", cvec, ae**2)
        i_fin = HSTEP * (NSTEP * sum_c - ptr)
        out[r0 : r0 + BSH, :D] = y_fin.astype(np.float32)
        out[r0 : r0 + BSH, D] = i_fin.astype(np.float32)

    if _trace:
        return out.astype(in_dtype, copy=False), res
    return out.astype(in_dtype, copy=False)
